# revision 9
# baseline (speedup 1.0000x reference)
"""Masked attention (out, p_attn) on 8 Trainium2 NeuronCores.

Problem shapes: Q,K,V [B=2, H=16, S=2048, D=64] f32, mask [B,1,1,S] int32.
Returns (out [B,H,S,D], p_attn [B,H,S,S]) both f32, matching

    scores = (Q @ K^T) / sqrt(D);  scores[mask==0] = -1e9
    p_attn = softmax(scores, axis=-1);  out = p_attn @ V

Sharding: the 32 (b,h) pairs are split 4-per-core across 8 cores (pure
data/head parallelism, no collectives).

Device kernel (k-major so the softmax reduction rides the matmul
contraction axis):
  S^T[k,q]  = matmul(lhsT=K^T[64,128k], rhs=Q^T[64,512q])     (float32r)
  E^T       = exp(S^T * (1/sqrt(D)) + bias_k)      bias_k = (mask_k-1)*1e9
  [outT|den]= matmul(lhsT=[V|1|0][128k,66], rhs=E^T[128k,512q]) acc over k
  P^T       = E^T * (1/den)[q]  -> HBM   ;  outT * (1/den)[q] -> HBM

Perf structure:
  - Two (b,h) pairs are row-packed into the 128-row PE array (QK uses only
    K=64 contraction), so their QK matmuls run concurrently in disjoint
    row-groups (lhsT base partitions 0 and 64 -> tile_position rows).
  - exp folds mask+scale via per-partition bias; no row-max pass is needed
    (scores are ~N(0,1), exp can't overflow; masked lanes hit exp(-1e9)=0
    exactly like the reference).
  - The ones-column of V makes the PV matmul emit softmax denominators.
  - E is written as float32r (tfloat32-rounded) by the ACT so the PV
    matmul can stream it at full fp32r rate with no extra cast pass.
  - Normalize muls are split DVE/GpSimd; denominator reciprocal uses the
    fast custom-DVE op; P^T leaves as one linear 4 MB DMA per (pair,qc)
    into a blocked DRAM layout the host unscrambles during the gather.
"""

import math

import numpy as np

import concourse.bacc as bacc
import concourse.mybir as mybir
import concourse.tile as tile
from concourse import bass_utils

B, H, S, D = 2, 16, 2048, 64
N_CORES = 8
PAIRS = B * H
PPC = PAIRS // N_CORES  # pairs per core
NG = PPC // 2           # packed pair-pairs per core
QCH = 512               # q-chunk width (max fp32 matmul free dim)
NQC = S // QCH
KCH = 128               # k-chunk height (matmul output partitions)
NKC = S // KCH
VPAD = D + 2            # V columns padded: [V | 1 | 0] (f32r needs even M)
SCALE = 1.0 / math.sqrt(D)
F32 = mybir.dt.float32
F32R = mybir.dt.float32r
# which k-chunks' normalize-mul runs on GpSimd instead of DVE (GpSimd is
# ~2x slower per op; 5/16 there balances the two engines)
GPS_KC = frozenset((2, 5, 8, 11, 14))

_NC_CACHE = None


def _build_nc():
    nc = bacc.Bacc("TRN2", target_bir_lowering=False, debug=False, num_devices=1)
    # packed transposed Q/K: per pair-pair g, rows 0:64 = pair 2g, 64:128 = 2g+1
    qt = nc.dram_tensor("qt", [NG, KCH, S], F32R, kind="ExternalInput").ap()
    kt = nc.dram_tensor("kt", [NG, KCH, S], F32R, kind="ExternalInput").ap()
    vp = nc.dram_tensor("vp", [PPC, KCH, NKC, VPAD], F32R, kind="ExternalInput").ap()
    bias = nc.dram_tensor("bias", [PPC, KCH, NKC], F32, kind="ExternalInput").ap()
    # blocked outputs: host unscrambles (qc, kp, kc, q) -> [q, k]
    pT = nc.dram_tensor("pT", [PPC, NQC, KCH, NKC, QCH], F32R, kind="ExternalOutput").ap()
    oT = nc.dram_tensor("oT", [PPC, NQC, D, QCH], F32, kind="ExternalOutput").ap()

    Exp = mybir.ActivationFunctionType.Exp

    with tile.TileContext(nc) as tc:
        with (
            tc.tile_pool(name="inp", bufs=2) as inp_pool,
            tc.tile_pool(name="e", bufs=3) as e_pool,
            tc.tile_pool(name="small", bufs=3) as small_pool,
            tc.tile_pool(name="qk_ps", bufs=4, space="PSUM") as qk_pool,
            tc.tile_pool(name="pv_ps", bufs=4, space="PSUM") as pv_pool,
        ):
            # Prologue: load every pair-group's inputs up front (inp_pool
            # bufs=2 covers both groups). Keeping these off the tail of the
            # output-DMA queue matters: Sync issues DMAs in order, so input
            # loads queued behind data-dependent output DMAs starve the PE
            # at group boundaries.
            gin = []
            for g in range(NG):
                qt_sb = inp_pool.tile([KCH, S], F32R, tag="qt", name=f"qt{g}")
                nc.sync.dma_start(out=qt_sb, in_=qt[g])
                kt_sb = inp_pool.tile([KCH, S], F32R, tag="kt", name=f"kt{g}")
                nc.sync.dma_start(out=kt_sb, in_=kt[g])
                vp_sb = []
                bias_sb = []
                for s in range(2):
                    vt = inp_pool.tile([KCH, NKC, VPAD], F32R, tag=f"vp{s}",
                                       name=f"vp{g}{s}")
                    nc.sync.dma_start(out=vt, in_=vp[2 * g + s])
                    vp_sb.append(vt)
                    bt = inp_pool.tile([KCH, NKC], F32, tag=f"bias{s}",
                                       name=f"bias{g}{s}")
                    nc.sync.dma_start(out=bt, in_=bias[2 * g + s])
                    bias_sb.append(bt)
                gin.append((qt_sb, kt_sb, vp_sb, bias_sb))

            for g in range(NG):
                qt_sb, kt_sb, vp_sb, bias_sb = gin[g]
                for qc in range(NQC):
                    qs = slice(qc * QCH, (qc + 1) * QCH)
                    e_sb = [e_pool.tile([KCH, NKC, QCH], F32R, tag="e", name=f"e{s}")
                            for s in range(2)]
                    pv_ps = [[pv_pool.tile([VPAD, QCH], F32, tag="pv",
                                            name=f"pv{s}{h}")
                              for h in range(2)] for s in range(2)]

                    def pv_mm(s, kc):
                        # fp32r K=128 is ~7x slower than K=64 (the 4-byte
                        # operand halves the effective contraction depth),
                        # so each PV is two concurrent K=64 row-tiles into
                        # separate banks, summed at readout.
                        for h in range(2):
                            rows = slice(64 * h, 64 * h + 64)
                            nc.tensor.matmul(
                                pv_ps[s][h],
                                lhsT=vp_sb[s][rows, kc, :],
                                rhs=e_sb[s][rows, kc, :],
                                start=(kc == 0),
                                stop=(kc == NKC - 1),
                                tile_position=(64 * h, 0),
                            )

                    # Software-pipelined by one k-chunk: PE issues the two
                    # row-packed QK(kc) then PV(kc-1), so it never stalls
                    # on the ACT exp of the chunk it just produced.
                    for kc in range(NKC):
                        ks = slice(kc * KCH, (kc + 1) * KCH)
                        qk = [qk_pool.tile([KCH, QCH], F32, tag="qk", name=f"qk{s}")
                              for s in range(2)]
                        for s in range(2):
                            rows = slice(64 * s, 64 * s + 64)
                            nc.tensor.matmul(
                                qk[s],
                                lhsT=kt_sb[rows, ks],
                                rhs=qt_sb[rows, qs],
                                start=True,
                                stop=True,
                                tile_position=(64 * s, 0),
                            )
                        for s in range(2):
                            nc.scalar.activation(
                                e_sb[s][:, kc, :],
                                qk[s],
                                Exp,
                                bias=bias_sb[s][:, kc:kc + 1],
                                scale=SCALE,
                            )
                        if kc > 0:
                            pv_mm(0, kc - 1)
                            pv_mm(1, kc - 1)
                    pv_mm(0, NKC - 1)
                    pv_mm(1, NKC - 1)

                    for s in range(2):
                        p = 2 * g + s
                        p1c = small_pool.tile([VPAD, QCH], F32, tag="p1c")
                        nc.vector.tensor_copy(p1c, pv_ps[s][1])
                        den = small_pool.tile([1, QCH], F32, tag="den")
                        nc.vector.tensor_add(
                            den, pv_ps[s][0][D:D + 1, :], p1c[D:D + 1, :])
                        rinv = small_pool.tile([1, QCH], F32, tag="rinv")
                        nc.vector.reciprocal_approx_fast(rinv, den)
                        r128 = small_pool.tile([KCH, QCH], F32, tag="r128")
                        nc.gpsimd.partition_broadcast(r128, rinv)
                        osum = small_pool.tile([D, QCH], F32, tag="osum")
                        nc.vector.tensor_add(
                            osum, pv_ps[s][0][0:D, :], p1c[0:D, :])
                        o_sb = small_pool.tile([D, QCH], F32, tag="o")
                        nc.vector.tensor_mul(o_sb, osum, r128[0:D, :])
                        nc.sync.dma_start(out=oT[p, qc], in_=o_sb)
                        for kc in range(NKC):
                            eng = nc.gpsimd if kc in GPS_KC else nc.vector
                            ekc = e_sb[s][:, kc, :]
                            eng.tensor_mul(ekc, ekc, r128)
                        nc.sync.dma_start(out=pT[p, qc], in_=e_sb[s][:])
    nc.finalize()
    return nc


def _get_nc():
    global _NC_CACHE
    if _NC_CACHE is None:
        _NC_CACHE = _build_nc()
    return _NC_CACHE


def _prep_core_inputs(q, k, v, mask, core):
    qt = np.empty((NG, KCH, S), np.float32)
    kt = np.empty((NG, KCH, S), np.float32)
    vp = np.zeros((PPC, KCH, NKC, VPAD), np.float32)
    bias = np.empty((PPC, KCH, NKC), np.float32)
    for i in range(PPC):
        idx = core * PPC + i
        b, h = idx // H, idx % H
        g, s = i // 2, i % 2
        qt[g, 64 * s:64 * s + 64] = q[b, h].T
        kt[g, 64 * s:64 * s + 64] = k[b, h].T
        # [S, D] -> [NKC, KCH, D] -> [KCH, NKC, D], plus the ones column
        vp[i, :, :, :D] = v[b, h].reshape(NKC, KCH, D).transpose(1, 0, 2)
        vp[i, :, :, D] = 1.0
        m = mask[b, 0, 0].astype(np.float32)  # [S]
        bias[i] = ((m - 1.0) * 1e9).reshape(NKC, KCH).T
    return {"qt": qt, "kt": kt, "vp": vp, "bias": bias}


def run_sharded(q, k, v, mask, trace=False, tmpdir=None, trace_cores=None):
    """Run the device kernel; returns (out, p_attn, BassKernelResults)."""
    nc = _get_nc()
    in_maps = [_prep_core_inputs(q, k, v, mask, c) for c in range(N_CORES)]
    res = bass_utils.run_bass_kernel_spmd(
        nc, in_maps, core_ids=list(range(N_CORES)), trace=trace, tmpdir=tmpdir,
        trace_cores=trace_cores,
    )
    out = np.empty((B, H, S, D), np.float32)
    p_attn = np.empty((B, H, S, S), np.float32)
    for c in range(N_CORES):
        r = res.results[c]
        for i in range(PPC):
            idx = c * PPC + i
            b, h = idx // H, idx % H
            # oT blocked [NQC, D, QCH] -> [S, D]
            out[b, h] = r["oT"][i].transpose(0, 2, 1).reshape(S, D)
            # pT blocked [NQC, KCH, NKC, QCH]: [qc, kp, kc, qq] -> [q, k]
            p_attn[b, h] = (
                r["pT"][i].transpose(0, 3, 2, 1).reshape(S, S)
            )
    return out, p_attn, res


def kernel(query, key, value, mask):
    q = np.asarray(query, np.float32)
    k = np.asarray(key, np.float32)
    v = np.asarray(value, np.float32)
    m = np.asarray(mask)
    out, p_attn, _ = run_sharded(q, k, v, m)
    return out, p_attn


# revision 10
# speedup vs baseline: 1.1228x; 1.1228x over previous
"""Masked attention (out, p_attn) on 8 Trainium2 NeuronCores.

Problem shapes: Q,K,V [B=2, H=16, S=2048, D=64] f32, mask [B,1,1,S] int32.
Returns (out [B,H,S,D], p_attn [B,H,S,S]) both f32, matching

    scores = (Q @ K^T) / sqrt(D);  scores[mask==0] = -1e9
    p_attn = softmax(scores, axis=-1);  out = p_attn @ V

Sharding: the 32 (b,h) pairs are split 4-per-core across 8 cores (pure
data/head parallelism, no collectives).

Device kernel (k-major so the softmax reduction rides the matmul
contraction axis):
  S^T[k,q]  = matmul(lhsT=K^T[64,128k], rhs=Q^T[64,512q])     (float32r)
  E^T       = exp(S^T * (1/sqrt(D)) + bias_k)      bias_k = (mask_k-1)*1e9
  [outT|den]= matmul(lhsT=[V|1|0][128k,66], rhs=E^T[128k,512q]) acc over k
  P^T       = E^T * (1/den)[q]  -> HBM   ;  outT * (1/den)[q] -> HBM

Perf structure:
  - Two (b,h) pairs are row-packed into the 128-row PE array (QK uses only
    K=64 contraction), so their QK matmuls run concurrently in disjoint
    row-groups (lhsT base partitions 0 and 64 -> tile_position rows).
  - exp folds mask+scale via per-partition bias; no row-max pass is needed
    (scores are ~N(0,1), exp can't overflow; masked lanes hit exp(-1e9)=0
    exactly like the reference).
  - The ones-column of V makes the PV matmul emit softmax denominators.
  - E is written as float32r (tfloat32-rounded) by the ACT so the PV
    matmul can stream it at full fp32r rate with no extra cast pass.
  - Normalize muls are split DVE/GpSimd; denominator reciprocal uses the
    fast custom-DVE op; P^T leaves as one linear 4 MB DMA per (pair,qc)
    into a blocked DRAM layout the host unscrambles during the gather.
"""

import math

import numpy as np

import concourse.bacc as bacc
import concourse.mybir as mybir
import concourse.tile as tile
from concourse import bass_utils

B, H, S, D = 2, 16, 2048, 64
N_CORES = 8
PAIRS = B * H
PPC = PAIRS // N_CORES  # pairs per core
NG = PPC // 2           # packed pair-pairs per core
QCH = 512               # q-chunk width (max fp32 matmul free dim)
NQC = S // QCH
KCH = 128               # k-chunk height (matmul output partitions)
NKC = S // KCH
VPAD = D + 2            # V columns padded: [V | 1 | 0] (f32r needs even M)
SCALE = 1.0 / math.sqrt(D)
F32 = mybir.dt.float32
F32R = mybir.dt.float32r
# which k-chunks' normalize-mul runs on GpSimd instead of DVE (GpSimd is
# ~2x slower per op; 5/16 there balances the two engines)
GPS_KC = frozenset((2, 5, 8, 11, 14))

_NC_CACHE = None


def _build_nc():
    nc = bacc.Bacc("TRN2", target_bir_lowering=False, debug=False, num_devices=1)
    # packed transposed Q/K: per pair-pair g, rows 0:64 = pair 2g, 64:128 = 2g+1
    qt = nc.dram_tensor("qt", [NG, KCH, S], F32R, kind="ExternalInput").ap()
    kt = nc.dram_tensor("kt", [NG, KCH, S], F32R, kind="ExternalInput").ap()
    vp = nc.dram_tensor("vp", [PPC, KCH, NKC, VPAD], F32R, kind="ExternalInput").ap()
    bias = nc.dram_tensor("bias", [PPC, KCH, NKC], F32, kind="ExternalInput").ap()
    # blocked outputs: host unscrambles (qc, kp, kc, q) -> [q, k]
    pT = nc.dram_tensor("pT", [PPC, NQC, KCH, NKC, QCH], F32R, kind="ExternalOutput").ap()
    oT = nc.dram_tensor("oT", [PPC, NQC, D, QCH], F32, kind="ExternalOutput").ap()

    Exp = mybir.ActivationFunctionType.Exp

    with tile.TileContext(nc) as tc:
        with (
            tc.tile_pool(name="inp", bufs=2) as inp_pool,
            tc.tile_pool(name="e", bufs=6) as e_pool,
            tc.tile_pool(name="small", bufs=2) as small_pool,
            tc.tile_pool(name="qk_ps", bufs=4, space="PSUM") as qk_pool,
            tc.tile_pool(name="pv_ps", bufs=4, space="PSUM") as pv_pool,
        ):
            # Prologue: load every pair-group's inputs up front (inp_pool
            # bufs=2 covers both groups). Keeping these off the tail of the
            # output-DMA queue matters: Sync issues DMAs in order, so input
            # loads queued behind data-dependent output DMAs starve the PE
            # at group boundaries.
            gin = []
            for g in range(NG):
                qt_sb = inp_pool.tile([KCH, S], F32R, tag="qt", name=f"qt{g}")
                nc.sync.dma_start(out=qt_sb, in_=qt[g])
                kt_sb = inp_pool.tile([KCH, S], F32R, tag="kt", name=f"kt{g}")
                nc.sync.dma_start(out=kt_sb, in_=kt[g])
                vp_sb = []
                bias_sb = []
                for s in range(2):
                    vt = inp_pool.tile([KCH, NKC, VPAD], F32R, tag=f"vp{s}",
                                       name=f"vp{g}{s}")
                    nc.sync.dma_start(out=vt, in_=vp[2 * g + s])
                    vp_sb.append(vt)
                    bt = inp_pool.tile([KCH, NKC], F32, tag=f"bias{s}",
                                       name=f"bias{g}{s}")
                    nc.sync.dma_start(out=bt, in_=bias[2 * g + s])
                    bias_sb.append(bt)
                gin.append((qt_sb, kt_sb, vp_sb, bias_sb))

            for g in range(NG):
                qt_sb, kt_sb, vp_sb, bias_sb = gin[g]
                for qc in range(NQC):
                    qs = slice(qc * QCH, (qc + 1) * QCH)
                    HK = NKC // 2
                    e_sb = [[e_pool.tile([KCH, HK, QCH], F32R, tag="e",
                                         name=f"e{s}h{h}") for h in range(2)]
                            for s in range(2)]

                    def e_ap(s, kc):
                        return e_sb[s][kc // HK][:, kc % HK, :]
                    pv_ps = [[pv_pool.tile([VPAD, QCH], F32, tag="pv",
                                            name=f"pv{s}{h}")
                              for h in range(2)] for s in range(2)]

                    def pv_mm(s, kc):
                        # fp32r K=128 is ~7x slower than K=64 (the 4-byte
                        # operand halves the effective contraction depth),
                        # so each PV is two concurrent K=64 row-tiles into
                        # separate banks, summed at readout.
                        for h in range(2):
                            rows = slice(64 * h, 64 * h + 64)
                            nc.tensor.matmul(
                                pv_ps[s][h],
                                lhsT=vp_sb[s][rows, kc, :],
                                rhs=e_ap(s, kc)[rows, :],
                                start=(kc == 0),
                                stop=(kc == NKC - 1),
                                tile_position=(64 * h, 0),
                            )

                    # Software-pipelined by one k-chunk: PE issues the two
                    # row-packed QK(kc) then PV(kc-1), so it never stalls
                    # on the ACT exp of the chunk it just produced.
                    for kc in range(NKC):
                        ks = slice(kc * KCH, (kc + 1) * KCH)
                        qk = [qk_pool.tile([KCH, QCH], F32, tag="qk", name=f"qk{s}")
                              for s in range(2)]
                        for s in range(2):
                            rows = slice(64 * s, 64 * s + 64)
                            nc.tensor.matmul(
                                qk[s],
                                lhsT=kt_sb[rows, ks],
                                rhs=qt_sb[rows, qs],
                                start=True,
                                stop=True,
                                tile_position=(64 * s, 0),
                            )
                        for s in range(2):
                            nc.scalar.activation(
                                e_ap(s, kc),
                                qk[s],
                                Exp,
                                bias=bias_sb[s][:, kc:kc + 1],
                                scale=SCALE,
                            )
                        if kc > 0:
                            pv_mm(0, kc - 1)
                            pv_mm(1, kc - 1)
                    pv_mm(0, NKC - 1)
                    pv_mm(1, NKC - 1)

                    for s in range(2):
                        p = 2 * g + s
                        p1c = small_pool.tile([VPAD, QCH], F32, tag="p1c")
                        nc.vector.tensor_copy(p1c, pv_ps[s][1])
                        den = small_pool.tile([1, QCH], F32, tag="den")
                        nc.vector.tensor_add(
                            den, pv_ps[s][0][D:D + 1, :], p1c[D:D + 1, :])
                        rinv = small_pool.tile([1, QCH], F32, tag="rinv")
                        nc.vector.reciprocal_approx_fast(rinv, den)
                        r128 = small_pool.tile([KCH, QCH], F32, tag="r128")
                        nc.gpsimd.partition_broadcast(r128, rinv)
                        osum = small_pool.tile([D, QCH], F32, tag="osum")
                        nc.vector.tensor_add(
                            osum, pv_ps[s][0][0:D, :], p1c[0:D, :])
                        o_sb = small_pool.tile([D, QCH], F32, tag="o")
                        nc.vector.tensor_mul(o_sb, osum, r128[0:D, :])
                        nc.sync.dma_start(out=oT[p, qc], in_=o_sb)
                        for h in range(2):
                            for kc in range(h * HK, (h + 1) * HK):
                                eng = nc.gpsimd if kc in GPS_KC else nc.vector
                                ekc = e_ap(s, kc)
                                eng.tensor_mul(ekc, ekc, r128)
                            nc.sync.dma_start(
                                out=pT[p, qc][:, h * HK:(h + 1) * HK, :],
                                in_=e_sb[s][h][:],
                            )
    nc.finalize()
    return nc


def _get_nc():
    global _NC_CACHE
    if _NC_CACHE is None:
        _NC_CACHE = _build_nc()
    return _NC_CACHE


def _prep_core_inputs(q, k, v, mask, core):
    qt = np.empty((NG, KCH, S), np.float32)
    kt = np.empty((NG, KCH, S), np.float32)
    vp = np.zeros((PPC, KCH, NKC, VPAD), np.float32)
    bias = np.empty((PPC, KCH, NKC), np.float32)
    for i in range(PPC):
        idx = core * PPC + i
        b, h = idx // H, idx % H
        g, s = i // 2, i % 2
        qt[g, 64 * s:64 * s + 64] = q[b, h].T
        kt[g, 64 * s:64 * s + 64] = k[b, h].T
        # [S, D] -> [NKC, KCH, D] -> [KCH, NKC, D], plus the ones column
        vp[i, :, :, :D] = v[b, h].reshape(NKC, KCH, D).transpose(1, 0, 2)
        vp[i, :, :, D] = 1.0
        m = mask[b, 0, 0].astype(np.float32)  # [S]
        bias[i] = ((m - 1.0) * 1e9).reshape(NKC, KCH).T
    return {"qt": qt, "kt": kt, "vp": vp, "bias": bias}


def run_sharded(q, k, v, mask, trace=False, tmpdir=None, trace_cores=None):
    """Run the device kernel; returns (out, p_attn, BassKernelResults)."""
    nc = _get_nc()
    in_maps = [_prep_core_inputs(q, k, v, mask, c) for c in range(N_CORES)]
    res = bass_utils.run_bass_kernel_spmd(
        nc, in_maps, core_ids=list(range(N_CORES)), trace=trace, tmpdir=tmpdir,
        trace_cores=trace_cores,
    )
    out = np.empty((B, H, S, D), np.float32)
    p_attn = np.empty((B, H, S, S), np.float32)
    for c in range(N_CORES):
        r = res.results[c]
        for i in range(PPC):
            idx = c * PPC + i
            b, h = idx // H, idx % H
            # oT blocked [NQC, D, QCH] -> [S, D]
            out[b, h] = r["oT"][i].transpose(0, 2, 1).reshape(S, D)
            # pT blocked [NQC, KCH, NKC, QCH]: [qc, kp, kc, qq] -> [q, k]
            p_attn[b, h] = (
                r["pT"][i].transpose(0, 3, 2, 1).reshape(S, S)
            )
    return out, p_attn, res


def kernel(query, key, value, mask):
    q = np.asarray(query, np.float32)
    k = np.asarray(key, np.float32)
    v = np.asarray(value, np.float32)
    m = np.asarray(mask)
    out, p_attn, _ = run_sharded(q, k, v, m)
    return out, p_attn


# revision 11
# speedup vs baseline: 1.2843x; 1.1438x over previous
"""Masked attention (out, p_attn) on 8 Trainium2 NeuronCores.

Problem shapes: Q,K,V [B=2, H=16, S=2048, D=64] f32, mask [B,1,1,S] int32.
Returns (out [B,H,S,D], p_attn [B,H,S,S]) both f32, matching

    scores = (Q @ K^T) / sqrt(D);  scores[mask==0] = -1e9
    p_attn = softmax(scores, axis=-1);  out = p_attn @ V

Sharding: the 32 (b,h) pairs are split 4-per-core across 8 cores (pure
data/head parallelism, no collectives).

Device kernel (k-major so the softmax reduction rides the matmul
contraction axis):
  S^T[k,q]  = matmul(lhsT=K^T[64,128k], rhs=Q^T[64,512q])     (float32r)
  E^T       = exp(S^T * (1/sqrt(D)) + bias_k)      bias_k = (mask_k-1)*1e9
  [outT|den]= matmul(lhsT=[V|1|0][128k,66], rhs=E^T[128k,512q]) acc over k
  P^T       = E^T * (1/den)[q]  -> HBM   ;  outT * (1/den)[q] -> HBM

Perf structure:
  - Two (b,h) pairs are row-packed into the 128-row PE array (QK uses only
    K=64 contraction), so their QK matmuls run concurrently in disjoint
    row-groups (lhsT base partitions 0 and 64 -> tile_position rows).
  - exp folds mask+scale via per-partition bias; no row-max pass is needed
    (scores are ~N(0,1), exp can't overflow; masked lanes hit exp(-1e9)=0
    exactly like the reference).
  - The ones-column of V makes the PV matmul emit softmax denominators.
  - E is written as float32r (tfloat32-rounded) by the ACT so the PV
    matmul can stream it at full fp32r rate with no extra cast pass.
  - Normalize muls are split DVE/GpSimd; denominator reciprocal uses the
    fast custom-DVE op; P^T leaves as one linear 4 MB DMA per (pair,qc)
    into a blocked DRAM layout the host unscrambles during the gather.
"""

import math

import numpy as np

import concourse.bacc as bacc
import concourse.mybir as mybir
import concourse.tile as tile
from concourse import bass_utils

B, H, S, D = 2, 16, 2048, 64
N_CORES = 8
PAIRS = B * H
PPC = PAIRS // N_CORES  # pairs per core
NG = PPC // 2           # packed pair-pairs per core
QCH = 512               # q-chunk width (max fp32 matmul free dim)
NQC = S // QCH
KCH = 128               # k-chunk height (matmul output partitions)
NKC = S // KCH
VPAD = D + 2            # V columns padded: [V | 1 | 0] (f32r needs even M)
SCALE = 1.0 / math.sqrt(D)
F32 = mybir.dt.float32
F32R = mybir.dt.float32r
# which k-chunks' normalize-mul runs on GpSimd instead of DVE (GpSimd is
# ~2x slower per op; 5/16 there balances the two engines)
GPS_KC = frozenset((2, 5, 8, 11, 14))

_NC_CACHE = {}


def _build_nc(nkc_act):
    nc = bacc.Bacc("TRN2", target_bir_lowering=False, debug=False, num_devices=1)
    # packed transposed Q/K: per pair-pair g, rows 0:64 = pair 2g, 64:128 = 2g+1
    qt = nc.dram_tensor("qt", [NG, KCH, S], F32R, kind="ExternalInput").ap()
    kt = nc.dram_tensor("kt", [NG, KCH, S], F32R, kind="ExternalInput").ap()
    vp = nc.dram_tensor("vp", [PPC, KCH, NKC, VPAD], F32R, kind="ExternalInput").ap()
    bias = nc.dram_tensor("bias", [PPC, KCH, NKC], F32, kind="ExternalInput").ap()
    # blocked outputs: host unscrambles (qc, kp, kc, q) -> [q, k]
    pT = nc.dram_tensor("pT", [PPC, NQC, KCH, NKC, QCH], F32R, kind="ExternalOutput").ap()
    oT = nc.dram_tensor("oT", [PPC, NQC, D, QCH], F32, kind="ExternalOutput").ap()

    Exp = mybir.ActivationFunctionType.Exp

    HK = nkc_act - nkc_act // 2   # low-half chunks (>= hi half)
    HK2 = nkc_act // 2
    with tile.TileContext(nc) as tc:
        with (
            tc.tile_pool(name="inp", bufs=2) as inp_pool,
            tc.tile_pool(name="e", bufs=6) as e_pool,
            tc.tile_pool(name="small", bufs=2) as small_pool,
            tc.tile_pool(name="qk_ps", bufs=4, space="PSUM") as qk_pool,
            tc.tile_pool(name="pv_ps", bufs=4, space="PSUM") as pv_pool,
        ):
            # Prologue: load every pair-group's inputs up front (inp_pool
            # bufs=2 covers both groups). Keeping these off the tail of the
            # output-DMA queue matters: Sync issues DMAs in order, so input
            # loads queued behind data-dependent output DMAs starve the PE
            # at group boundaries.
            gin = []
            for g in range(NG):
                qt_sb = inp_pool.tile([KCH, S], F32R, tag="qt", name=f"qt{g}")
                nc.sync.dma_start(out=qt_sb, in_=qt[g])
                kt_sb = inp_pool.tile([KCH, S], F32R, tag="kt", name=f"kt{g}")
                nc.sync.dma_start(out=kt_sb, in_=kt[g])
                vp_sb = []
                bias_sb = []
                for s in range(2):
                    vt = inp_pool.tile([KCH, NKC, VPAD], F32R, tag=f"vp{s}",
                                       name=f"vp{g}{s}")
                    nc.sync.dma_start(out=vt, in_=vp[2 * g + s])
                    vp_sb.append(vt)
                    bt = inp_pool.tile([KCH, NKC], F32, tag=f"bias{s}",
                                       name=f"bias{g}{s}")
                    nc.sync.dma_start(out=bt, in_=bias[2 * g + s])
                    bias_sb.append(bt)
                gin.append((qt_sb, kt_sb, vp_sb, bias_sb))

            for g in range(NG):
                qt_sb, kt_sb, vp_sb, bias_sb = gin[g]
                for qc in range(NQC):
                    qs = slice(qc * QCH, (qc + 1) * QCH)
                    e_sb = [[e_pool.tile([KCH, HK, QCH], F32R, tag="e",
                                         name=f"e{s}h{h}") for h in range(2)]
                            for s in range(2)]

                    def e_ap(s, kc):
                        return e_sb[s][kc // HK][:, kc % HK, :]
                    pv_ps = [[pv_pool.tile([VPAD, QCH], F32, tag="pv",
                                            name=f"pv{s}{h}")
                              for h in range(2)] for s in range(2)]

                    def pv_mm(s, kc):
                        # fp32r K=128 is ~7x slower than K=64 (the 4-byte
                        # operand halves the effective contraction depth),
                        # so each PV is two concurrent K=64 row-tiles into
                        # separate banks, summed at readout.
                        for h in range(2):
                            rows = slice(64 * h, 64 * h + 64)
                            nc.tensor.matmul(
                                pv_ps[s][h],
                                lhsT=vp_sb[s][rows, kc, :],
                                rhs=e_ap(s, kc)[rows, :],
                                start=(kc == 0),
                                stop=(kc == nkc_act - 1),
                                tile_position=(64 * h, 0),
                            )

                    # Software-pipelined by one k-chunk: PE issues the two
                    # row-packed QK(kc) then PV(kc-1), so it never stalls
                    # on the ACT exp of the chunk it just produced.
                    for kc in range(nkc_act):
                        ks = slice(kc * KCH, (kc + 1) * KCH)
                        qk = [qk_pool.tile([KCH, QCH], F32, tag="qk", name=f"qk{s}")
                              for s in range(2)]
                        for s in range(2):
                            rows = slice(64 * s, 64 * s + 64)
                            nc.tensor.matmul(
                                qk[s],
                                lhsT=kt_sb[rows, ks],
                                rhs=qt_sb[rows, qs],
                                start=True,
                                stop=True,
                                tile_position=(64 * s, 0),
                            )
                        for s in range(2):
                            nc.scalar.activation(
                                e_ap(s, kc),
                                qk[s],
                                Exp,
                                bias=bias_sb[s][:, kc:kc + 1],
                                scale=SCALE,
                            )
                        if kc > 0:
                            pv_mm(0, kc - 1)
                            pv_mm(1, kc - 1)
                    pv_mm(0, nkc_act - 1)
                    pv_mm(1, nkc_act - 1)

                    for s in range(2):
                        p = 2 * g + s
                        p1c = small_pool.tile([VPAD, QCH], F32, tag="p1c")
                        nc.vector.tensor_copy(p1c, pv_ps[s][1])
                        den = small_pool.tile([1, QCH], F32, tag="den")
                        nc.vector.tensor_add(
                            den, pv_ps[s][0][D:D + 1, :], p1c[D:D + 1, :])
                        rinv = small_pool.tile([1, QCH], F32, tag="rinv")
                        nc.vector.reciprocal_approx_fast(rinv, den)
                        r128 = small_pool.tile([KCH, QCH], F32, tag="r128")
                        nc.gpsimd.partition_broadcast(r128, rinv)
                        osum = small_pool.tile([D, QCH], F32, tag="osum")
                        nc.vector.tensor_add(
                            osum, pv_ps[s][0][0:D, :], p1c[0:D, :])
                        o_sb = small_pool.tile([D, QCH], F32, tag="o")
                        nc.vector.tensor_mul(o_sb, osum, r128[0:D, :])
                        nc.sync.dma_start(out=oT[p, qc], in_=o_sb)
                        for h, nh in ((0, HK), (1, HK2)):
                            if nh == 0:
                                continue
                            for j in range(nh):
                                kc = h * HK + j
                                eng = nc.gpsimd if kc % 3 == 2 else nc.vector
                                ekc = e_ap(s, kc)
                                eng.tensor_mul(ekc, ekc, r128)
                            nc.sync.dma_start(
                                out=pT[p, qc][:, h * HK:h * HK + nh, :],
                                in_=e_sb[s][h][:, 0:nh, :],
                            )
    nc.finalize()
    return nc


def _get_nc(nkc_act):
    if nkc_act not in _NC_CACHE:
        _NC_CACHE[nkc_act] = _build_nc(nkc_act)
    return _NC_CACHE[nkc_act]


def _key_perms(mask):
    """Per-batch permutation putting unmasked keys first, and the active
    k-chunk count (static loop bound shared by all cores/pairs)."""
    perms = []
    n_max = 1
    for b in range(B):
        m = np.asarray(mask[b, 0, 0]) != 0
        perm = np.argsort(~m, kind="stable")  # unmasked first, order kept
        perms.append(perm)
        n_max = max(n_max, int(m.sum()))
    nkc_act = min(NKC, (n_max + KCH - 1) // KCH)
    return perms, nkc_act


def _prep_core_inputs(q, k, v, mask, core, perms):
    qt = np.empty((NG, KCH, S), np.float32)
    kt = np.empty((NG, KCH, S), np.float32)
    vp = np.zeros((PPC, KCH, NKC, VPAD), np.float32)
    bias = np.empty((PPC, KCH, NKC), np.float32)
    for i in range(PPC):
        idx = core * PPC + i
        b, h = idx // H, idx % H
        g, s = i // 2, i % 2
        perm = perms[b]
        qt[g, 64 * s:64 * s + 64] = q[b, h].T
        kt[g, 64 * s:64 * s + 64] = k[b, h][perm].T
        # [S, D] -> [NKC, KCH, D] -> [KCH, NKC, D], plus the ones column
        vp[i, :, :, :D] = v[b, h][perm].reshape(NKC, KCH, D).transpose(1, 0, 2)
        vp[i, :, :, D] = 1.0
        m = (np.asarray(mask[b, 0, 0])[perm] != 0).astype(np.float32)
        bias[i] = ((m - 1.0) * 1e9).reshape(NKC, KCH).T
    return {"qt": qt, "kt": kt, "vp": vp, "bias": bias}


def run_sharded(q, k, v, mask, trace=False, tmpdir=None, trace_cores=None):
    """Run the device kernel; returns (out, p_attn, BassKernelResults)."""
    perms, nkc_act = _key_perms(mask)
    nc = _get_nc(nkc_act)
    in_maps = [_prep_core_inputs(q, k, v, mask, c, perms) for c in range(N_CORES)]
    res = bass_utils.run_bass_kernel_spmd(
        nc, in_maps, core_ids=list(range(N_CORES)), trace=trace, tmpdir=tmpdir,
        trace_cores=trace_cores,
    )
    out = np.empty((B, H, S, D), np.float32)
    p_attn = np.zeros((B, H, S, S), np.float32)
    na = nkc_act * KCH
    for c in range(N_CORES):
        r = res.results[c]
        for i in range(PPC):
            idx = c * PPC + i
            b, h = idx // H, idx % H
            # oT blocked [NQC, D, QCH] -> [S, D]
            out[b, h] = r["oT"][i].transpose(0, 2, 1).reshape(S, D)
            # pT blocked [NQC, KCH, NKC, QCH]: [qc, kp, kc, qq] -> [q, k'].
            # Only the first nkc_act chunks were computed (permuted keys);
            # the rest of p_attn stays exactly 0, matching the reference's
            # masked exp(-1e9-max) underflow.
            pk = r["pT"][i][:, :, :nkc_act, :].transpose(0, 3, 2, 1)
            p_attn[b, h][:, perms[b][:na]] = pk.reshape(S, na)
    return out, p_attn, res


def kernel(query, key, value, mask):
    q = np.asarray(query, np.float32)
    k = np.asarray(key, np.float32)
    v = np.asarray(value, np.float32)
    m = np.asarray(mask)
    out, p_attn, _ = run_sharded(q, k, v, m)
    return out, p_attn
